# revision 19
# baseline (speedup 1.0000x reference)
"""Trainium2 Bass kernel for nn_DCell (hierarchical DCell-style GNN).

Sharding: subsystem-parallel across 8 NeuronCores. Each core owns 64 of the
512 leaf subsystems (16 groups of 4 leaves, block-diagonal matmuls with
K=128) and the 4 mid subsystems fed by exactly those leaves. BatchNorm batch
stats (full batch B=2048) are therefore fully local to a core for the leaf
and mid layers; the BN affine is folded into the *next* layer's weights so
no full-size normalization pass over activations is ever needed. The root
layer is computed as per-core partial pre-activations (each core contributes
its 4 mids' features + a 16-row slice of the root gene input) that are
summed with a single AllReduce; every core then redundantly finishes the
root (tanh + full-batch BN) on the small [38, 2048] tensor.

Compute dtype is bf16 (inputs cast on host -> half the HBM traffic, full
TensorE rate); all normalization statistics and folds are fp32.

kernel(**inputs) takes full unsharded inputs, returns the full [2048, 38]
float32 output.
"""

import ml_dtypes
import numpy as np

import concourse.bass as bass
import concourse.mybir as mybir
import concourse.tile as tile
from concourse import bacc
from concourse import bass_utils

# Problem constants (hardcoded; kernel.py must be self-contained)
S, B, GL, OL = 512, 2048, 32, 20
M, C, GM, OM = 32, 16, 64, 20
GR, OR = 128, 38
EPS = 1e-5
NCORES = 8
LPC = S // NCORES      # 64 leaves per core
GPC = LPC // 4         # 16 leaf groups of 4 per core
MPC = M // NCORES      # 4 mids per core
BT = 512               # batch tile (free dim per matmul / psum bank)
NBT = B // BT          # 4

f32 = mybir.dt.float32
bf16 = mybir.dt.bfloat16
i32 = mybir.dt.int32
AF = mybir.ActivationFunctionType
ALU = mybir.AluOpType
NPBF16 = ml_dtypes.bfloat16

MAGIC = 0x5F3759DF  # fast inverse sqrt seed


def _emit_rsqrt(nc, sp, tag, out, a, magic_t, n):
    """out = 1/sqrt(a) elementwise on [P, n] fp32 tiles, DVE-only.

    Quake magic seed + 2 Newton iterations: rel err ~5e-6. a > 0.
    """
    P = a.shape[0]
    sh = sp.tile([P, n], i32, tag=f"{tag}sh", name=f"{tag}sh")
    nc.vector.tensor_scalar(sh, a.bitcast(i32), 1, None,
                            ALU.arith_shift_right)
    y0 = sp.tile([P, n], i32, tag=f"{tag}y0", name=f"{tag}y0")
    nc.vector.tensor_tensor(y0, magic_t[:P, 0:n], sh, ALU.subtract)
    y = y0.bitcast(f32)
    for it in range(2):
        # y <- y * (1.5 - 0.5*a*y*y)
        t1 = sp.tile([P, n], f32, tag=f"{tag}t1", name=f"{tag}t1_{it}")
        nc.vector.tensor_mul(t1, y, y)                    # y^2
        t2 = sp.tile([P, n], f32, tag=f"{tag}t2", name=f"{tag}t2_{it}")
        nc.vector.tensor_mul(t2, a, t1)                   # a*y^2
        t3 = sp.tile([P, n], f32, tag=f"{tag}t3", name=f"{tag}t3_{it}")
        nc.vector.tensor_scalar(t3, t2, -0.5, 1.5, ALU.mult, ALU.add)
        dst = out if it == 1 else sp.tile([P, n], f32, tag=f"{tag}y",
                                          name=f"{tag}y_{it}")
        nc.vector.tensor_mul(dst, y, t3)
        y = dst


def _build_nc():
    """Build (once) the SPMD Bass program run identically on all 8 cores."""
    nc = bacc.Bacc(
        "TRN2",
        target_bir_lowering=False,
        debug=False,
        enable_asserts=False,
        num_devices=NCORES,
    )

    # ---- per-core external I/O ----
    xleaf = nc.dram_tensor("xleaf", [GPC, 128, B], bf16, kind="ExternalInput").ap()
    wleaf = nc.dram_tensor("wleaf", [128, GPC * 80], bf16, kind="ExternalInput").ap()
    bleaf = nc.dram_tensor("bleaf", [80, GPC], f32, kind="ExternalInput").ap()
    gleaf = nc.dram_tensor("gleaf", [80, GPC], f32, kind="ExternalInput").ap()
    beleaf = nc.dram_tensor("beleaf", [80, GPC], f32, kind="ExternalInput").ap()
    xmid2 = nc.dram_tensor("xmid2", [2, 128, B], bf16, kind="ExternalInput").ap()
    wgmid = nc.dram_tensor("wgmid", [80, GPC * 80], bf16, kind="ExternalInput").ap()
    wxmid2 = nc.dram_tensor("wxmid2", [128, 2 * 80], bf16, kind="ExternalInput").ap()
    bmid = nc.dram_tensor("bmid", [80, 1], f32, kind="ExternalInput").ap()
    gmid = nc.dram_tensor("gmid", [80, 1], f32, kind="ExternalInput").ap()
    bemid = nc.dram_tensor("bemid", [80, 1], f32, kind="ExternalInput").ap()
    wcroot = nc.dram_tensor("wcroot", [80, OR], bf16, kind="ExternalInput").ap()
    wgroot = nc.dram_tensor("wgroot", [16, OR], bf16, kind="ExternalInput").ap()
    xroot = nc.dram_tensor("xroot", [16, B], bf16, kind="ExternalInput").ap()
    broot = nc.dram_tensor("broot", [OR, 1], f32, kind="ExternalInput").ap()
    groot = nc.dram_tensor("groot", [OR, 1], f32, kind="ExternalInput").ap()
    beroot = nc.dram_tensor("beroot", [OR, 1], f32, kind="ExternalInput").ap()
    y = nc.dram_tensor("y", [OR, B], f32, kind="ExternalOutput").ap()

    with tile.TileContext(nc) as tc:
        with (
            tc.tile_pool(name="const", bufs=1) as cp,
            tc.tile_pool(name="xp", bufs=6) as xp,
            tc.tile_pool(name="lt", bufs=16) as ltp,
            tc.tile_pool(name="small", bufs=2) as sp,
            tc.tile_pool(name="big", bufs=1) as bp,
            tc.tile_pool(name="psA", bufs=2, space="PSUM") as psA,
            tc.tile_pool(name="psM", bufs=4, space="PSUM") as psM,
            tc.tile_pool(name="psS", bufs=1, space="PSUM") as psS,
            tc.tile_pool(name="dram", bufs=1, space="DRAM") as dp,
        ):
            # ---- load constants/weights into SBUF ----
            wleaf_sb = cp.tile_from(wleaf)
            bleaf_sb = cp.tile_from(bleaf)
            gleaf_sb = cp.tile_from(gleaf)
            beleaf_sb = cp.tile_from(beleaf)
            wgmid_sb = cp.tile_from(wgmid)
            wxmid2_sb = cp.tile_from(wxmid2)
            xmid2a_sb = cp.tile_from(xmid2[0])
            xmid2b_sb = cp.tile_from(xmid2[1])
            xmid2_sb = [xmid2a_sb, xmid2b_sb]
            bmid_sb = cp.tile_from(bmid)
            gmid_sb = cp.tile_from(gmid)
            bemid_sb = cp.tile_from(bemid)
            wcroot_sb = cp.tile_from(wcroot)
            wgroot_sb = cp.tile_from(wgroot)
            xroot_sb = cp.tile_from(xroot)
            broot_sb = cp.tile_from(broot)
            groot_sb = cp.tile_from(groot)
            beroot_sb = cp.tile_from(beroot)

            magic_t = cp.tile([80, 4], i32, tag="magic", name="magict")
            nc.vector.memset(magic_t, MAGIC)
            zbias = cp.tile([80, 1], f32, tag="zbias", name="zbias")
            nc.vector.memset(zbias, 0.0)

            # persistent mid-accumulation psum banks (one per batch tile)
            mid_ps = [psM.tile([80, BT], f32, tag="mid", name=f"midps{b}")
                      for b in range(NBT)]
            # u = sum over all child features f of W_mid[f,:] * t_f (packed)
            u_ps = psS.tile([80, 1], f32, tag="tiny", name="ups")

            lt_tiles = []
            mv_cols = cp.tile([80, GPC, 2], f32, tag="mvall", name="mvall")
            stats_t = None
            for gi in range(GPC):
                mi, gj = gi // 4, gi % 4
                act_stats = (gj % 2 == 1)   # odd groups: stats via ACT accum
                ltile = ltp.tile([80, B], bf16, tag="lt", name=f"lt{gi}")
                lt_tiles.append(ltile)
                if gj == 0:
                    stats_t = sp.tile([80, 4, NBT, 6], f32, tag="st",
                                      name=f"st{mi}")
                if act_stats:
                    sumx = sp.tile([80, NBT], f32, tag="sx", name=f"sx{gi}")
                    sumq = sp.tile([80, NBT], f32, tag="sq", name=f"sq{gi}")
                # ---- leaf matmul + tanh (+ Sum accum) per batch tile ----
                xt = xp.tile([128, B], bf16, tag="x", name=f"x{gi}")
                nc.sync.dma_start(out=xt, in_=xleaf[gi])
                for bt in range(NBT):
                    ps = psA.tile([80, BT], f32, tag="leaf",
                                  name=f"lfps{gi}_{bt}")
                    nc.tensor.matmul(
                        ps[:, :],
                        wleaf_sb[:, 80 * gi:80 * gi + 80],
                        xt[:, bt * BT:(bt + 1) * BT],
                        start=True, stop=True)
                    nc.scalar.activation(
                        ltile[:, bt * BT:(bt + 1) * BT], ps[:, :],
                        AF.Tanh, bias=bleaf_sb[:, gi:gi + 1], scale=1.0,
                        accum_out=sumx[:, bt:bt + 1] if act_stats else None)
                # ---- leaf BN stats for this group ----
                if act_stats:
                    sqs = sp.tile([80, BT], bf16, tag="sqs", name=f"sqs{gi}")
                    for bt in range(NBT):
                        nc.scalar.activation(
                            sqs[:, :], ltile[:, bt * BT:(bt + 1) * BT],
                            AF.Square, bias=zbias[:, 0:1],
                            accum_out=sumq[:, bt:bt + 1])
                    # mean = sum(sumx)/B ; var = sum(sumq)/B - mean^2
                    sxr = sp.tile([80, 1], f32, tag="sxr", name=f"sxr{gi}")
                    nc.vector.tensor_reduce(
                        out=sxr, in_=sumx[:, :],
                        op=ALU.add, axis=mybir.AxisListType.X)
                    nc.vector.tensor_scalar_mul(
                        mv_cols[:, gi, 0:1], sxr, 1.0 / B)
                    mq = sp.tile([80, 1], f32, tag="mq2", name=f"mq2{gi}")
                    nc.vector.tensor_mul(mq, mv_cols[:, gi, 0:1],
                                         mv_cols[:, gi, 0:1])
                    sqr = sp.tile([80, 1], f32, tag="sqr", name=f"sqr{gi}")
                    nc.vector.tensor_reduce(
                        out=sqr, in_=sumq[:, :],
                        op=ALU.add, axis=mybir.AxisListType.X)
                    nc.vector.tensor_scalar(
                        mv_cols[:, gi, 1:2], sqr, 1.0 / B, mq,
                        ALU.mult, ALU.subtract)
                else:
                    for bt in range(NBT):
                        nc.vector.bn_stats(
                            out=stats_t[:, gj, bt, :],
                            in_=ltile[:, bt * BT:(bt + 1) * BT])
                    nc.vector.bn_aggr(out=mv_cols[:, gi, :],
                                      in_=stats_t[:, gj, :, :])

                if gj != 3:
                    continue
                # ---- mid mi: fold BN into child weights, accumulate ----
                mean4 = mv_cols[:, 4 * mi:4 * mi + 4, 0]
                var4 = mv_cols[:, 4 * mi:4 * mi + 4, 1]
                a4 = sp.tile([80, 4], f32, tag="a4", name=f"a4{mi}")
                nc.vector.tensor_scalar_add(a4, var4, EPS)
                rs4 = sp.tile([80, 4], f32, tag="rs4", name=f"rs4{mi}")
                _emit_rsqrt(nc, sp, "lf", rs4, a4, magic_t, 4)
                s_t = sp.tile([80, 4], f32, tag="s", name=f"s{mi}")
                nc.vector.tensor_mul(s_t, gleaf_sb[:, 4 * mi:4 * mi + 4], rs4)
                ms = sp.tile([80, 4], f32, tag="ms", name=f"ms{mi}")
                nc.vector.tensor_mul(ms, mean4, s_t)
                t_t = sp.tile([80, 4], bf16, tag="t", name=f"t{mi}")
                nc.vector.tensor_sub(t_t, beleaf_sb[:, 4 * mi:4 * mi + 4], ms)

                wft = sp.tile([80, 4 * 80], bf16, tag="wf", name=f"wf{mi}")
                for g2 in range(4):
                    idx = 4 * mi + g2
                    nc.vector.tensor_scalar_mul(
                        wft[:, 80 * g2:80 * g2 + 80],
                        wgmid_sb[:, 80 * idx:80 * idx + 80],
                        s_t[:, g2:g2 + 1])
                    nc.tensor.matmul(
                        u_ps[:, :],
                        wgmid_sb[:, 80 * idx:80 * idx + 80],
                        t_t[:, g2:g2 + 1],
                        start=(mi == 0 and g2 == 0),
                        stop=(mi == 3 and g2 == 3))
                for bt in range(NBT):
                    for g2 in range(4):
                        idx = 4 * mi + g2
                        nc.tensor.matmul(
                            mid_ps[bt][:, :],
                            wft[:, 80 * g2:80 * g2 + 80],
                            lt_tiles[idx][:, bt * BT:(bt + 1) * BT],
                            start=(mi == 0 and g2 == 0), stop=False)
                    if mi % 2 == 1:  # gene blocks for mid pair (mi-1, mi)
                        pr = mi // 2
                        nc.tensor.matmul(
                            mid_ps[bt][:, :],
                            wxmid2_sb[:, 80 * pr:80 * pr + 80],
                            xmid2_sb[pr][:, bt * BT:(bt + 1) * BT],
                            start=False, stop=(mi == 3))

            # ---- mid finish: bias (b_mid + u), tanh, BN stats ----
            vmid = sp.tile([80, 1], f32, tag="vmid", name="vmid")
            nc.vector.tensor_add(vmid, u_ps[:, :], bmid_sb[:, :])
            tmt = bp.tile([80, B], bf16, tag="tm", name="tmt")
            mst = sp.tile([80, NBT, 6], f32, tag="mst", name="mst")
            for bt in range(NBT):
                nc.scalar.activation(
                    tmt[:, bt * BT:(bt + 1) * BT], mid_ps[bt][:, :],
                    AF.Tanh, bias=vmid[:, 0:1], scale=1.0)
                nc.vector.bn_stats(out=mst[:, bt, :],
                                   in_=tmt[:, bt * BT:(bt + 1) * BT])
            mmv = sp.tile([80, 2], f32, tag="mmv", name="mmv")
            nc.vector.bn_aggr(out=mmv[:, :], in_=mst[:, :, :])
            ma = sp.tile([80, 1], f32, tag="ma", name="ma")
            nc.vector.tensor_scalar_add(ma, mmv[:, 1:2], EPS)
            mrs = sp.tile([80, 1], f32, tag="mrs", name="mrs")
            _emit_rsqrt(nc, sp, "md", mrs, ma, magic_t, 1)
            msm = sp.tile([80, 1], f32, tag="msm", name="msm")
            nc.vector.tensor_mul(msm, gmid_sb[:, :], mrs)
            mms = sp.tile([80, 1], f32, tag="mms", name="mms")
            nc.vector.tensor_mul(mms, mmv[:, 0:1], msm)
            mtm = sp.tile([80, 1], bf16, tag="mtm", name="mtm")
            nc.vector.tensor_sub(mtm, bemid_sb[:, :], mms)

            # fold mid BN into root child-weight block; const vec v_c
            wcf = sp.tile([80, OR], bf16, tag="wcf", name="wcf")
            nc.vector.tensor_scalar_mul(wcf, wcroot_sb[:, :], msm[:, 0:1])
            v_ps = psS.tile([OR, 1], f32, tag="tiny", name="vps")
            nc.tensor.matmul(v_ps[:, :], wcroot_sb[:, :], mtm[:, 0:1],
                             start=True, stop=True)
            v_sb = sp.tile([OR, 1], f32, tag="vsb", name="vsb")
            nc.vector.tensor_copy(v_sb, v_ps[:, :])

            # ---- root partial pre-activation [38, B] ----
            partial = bp.tile([OR, B], bf16, tag="prt", name="partial")
            for bt in range(NBT):
                psr = psA.tile([OR, BT], f32, tag="leaf", name=f"rtps{bt}")
                nc.tensor.matmul(
                    psr[:, :], wcf[:, :],
                    tmt[:, bt * BT:(bt + 1) * BT],
                    start=True, stop=False)
                nc.tensor.matmul(
                    psr[:, :], wgroot_sb[:, :],
                    xroot_sb[:, bt * BT:(bt + 1) * BT],
                    start=False, stop=True)
                nc.vector.tensor_scalar_add(
                    partial[:, bt * BT:(bt + 1) * BT], psr[:, :],
                    v_sb[:, 0:1])

            # ---- AllReduce the partial root pre-activation (bf16) ----
            cc_in = dp.tile([OR, B], bf16, tag="ccin", name="ccin")
            cc_out = dp.tile([OR, B], bf16, tag="ccout", name="ccout")
            nc.sync.dma_start(out=cc_in[:, :], in_=partial[:, :])
            nc.gpsimd.collective_compute(
                "AllReduce",
                ALU.add,
                replica_groups=[list(range(NCORES))],
                ins=[cc_in.opt()],
                outs=[cc_out.opt()],
            )
            rsum = bp.tile([OR, B], bf16, tag="prt2", name="rsum")
            nc.sync.dma_start(out=rsum[:, :], in_=cc_out[:, :])

            # ---- root finish: tanh, full-batch BN, write out ----
            rt = bp.tile([OR, B], bf16, tag="rt", name="rt")
            nc.scalar.activation(rt[:, :], rsum[:, :], AF.Tanh,
                                 bias=broot_sb[:, 0:1], scale=1.0)
            rst = sp.tile([OR, NBT, 6], f32, tag="rst", name="rst")
            for bt in range(NBT):
                nc.vector.bn_stats(out=rst[:, bt, :],
                                   in_=rt[:, bt * BT:(bt + 1) * BT])
            rmv = sp.tile([OR, 2], f32, tag="rmv", name="rmv")
            nc.vector.bn_aggr(out=rmv[:, :], in_=rst[:, :, :])
            ra = sp.tile([OR, 1], f32, tag="ra", name="ra")
            nc.vector.tensor_scalar_add(ra, rmv[:, 1:2], EPS)
            rrs = sp.tile([OR, 1], f32, tag="rrs", name="rrs")
            _emit_rsqrt(nc, sp, "rt", rrs, ra, magic_t, 1)
            rsc = sp.tile([OR, 1], f32, tag="rsc", name="rsc")
            nc.vector.tensor_mul(rsc, groot_sb[:, :], rrs)
            rms = sp.tile([OR, 1], f32, tag="rms", name="rms")
            nc.vector.tensor_mul(rms, rmv[:, 0:1], rsc)
            rsh = sp.tile([OR, 1], f32, tag="rsh", name="rsh")
            nc.vector.tensor_sub(rsh, beroot_sb[:, :], rms)
            ysb = bp.tile([OR, B], f32, tag="ysb", name="ysb")
            nc.vector.tensor_scalar(ysb[:, :], rt[:, :], rsc[:, 0:1],
                                    rsh[:, 0:1], ALU.mult, ALU.add)
            nc.sync.dma_start(out=y, in_=ysb[:, :])

    nc.compile()
    return nc


def _prep_in_maps(inputs):
    """Host-side sharding + layout prep (incl. bf16 cast). 8 in_maps."""
    f = np.float32
    x_leaf = np.asarray(inputs["x_leaf"], dtype=f)
    x_mid = np.asarray(inputs["x_mid"], dtype=f)
    x_root = np.asarray(inputs["x_root"], dtype=f)
    W_leaf = np.asarray(inputs["W_leaf"], dtype=f)
    b_leaf = np.asarray(inputs["b_leaf"], dtype=f)
    g_leaf = np.asarray(inputs["g_leaf"], dtype=f)
    be_leaf = np.asarray(inputs["be_leaf"], dtype=f)
    W_mid = np.asarray(inputs["W_mid"], dtype=f)
    b_mid = np.asarray(inputs["b_mid"], dtype=f)
    g_mid = np.asarray(inputs["g_mid"], dtype=f)
    be_mid = np.asarray(inputs["be_mid"], dtype=f)
    W_root = np.asarray(inputs["W_root"], dtype=f)
    b_root = np.asarray(inputs["b_root"], dtype=f)
    g_root = np.asarray(inputs["g_root"], dtype=f)
    be_root = np.asarray(inputs["be_root"], dtype=f)

    # gene-major leaf inputs, 4 leaves stacked per 128-partition group
    xleafT = np.ascontiguousarray(
        x_leaf.reshape(NCORES, GPC, 4, B, GL).transpose(0, 1, 2, 4, 3)
        .reshape(NCORES, GPC, 128, B)).astype(NPBF16)
    # mid gene inputs: per core, mid pairs (0,1) and (2,3) stacked to 128
    xmidT = (x_mid.reshape(NCORES, 2, 2, B, GM).transpose(0, 1, 2, 4, 3)
             .reshape(NCORES, 2, 128, B)).astype(NPBF16)
    xrootT = np.ascontiguousarray(x_root.T).astype(NPBF16)     # [128, B]

    in_maps = []
    for c in range(NCORES):
        d = {}
        d["xleaf"] = np.ascontiguousarray(xleafT[c])
        # block-diagonal leaf weights [128, 16*80]
        wl = np.zeros((128, GPC * 80), f)
        for gi in range(GPC):
            for j in range(4):
                s = LPC * c + 4 * gi + j
                wl[32 * j:32 * j + 32,
                   80 * gi + 20 * j:80 * gi + 20 * j + 20] = W_leaf[s]
        d["wleaf"] = wl.astype(NPBF16)
        for src, name in ((b_leaf, "bleaf"), (g_leaf, "gleaf"),
                          (be_leaf, "beleaf")):
            d[name] = np.ascontiguousarray(
                src[LPC * c:LPC * (c + 1)].reshape(GPC, 80).T)
        d["xmid2"] = np.ascontiguousarray(xmidT[c])
        wg = np.zeros((80, GPC * 80), f)
        # gene blocks for mid pairs: [128, 2*80]
        wx2 = np.zeros((128, 2 * 80), f)
        for mi in range(MPC):
            m = MPC * c + mi
            for gj in range(4):
                idx = 4 * mi + gj
                wg[:, 80 * idx + 20 * mi:80 * idx + 20 * mi + 20] = \
                    W_mid[m, GM + 80 * gj:GM + 80 * gj + 80, :]
            pr, sub = mi // 2, mi % 2
            wx2[64 * sub:64 * sub + 64,
                80 * pr + 20 * mi:80 * pr + 20 * mi + 20] = W_mid[m, :GM, :]
        d["wgmid"] = wg.astype(NPBF16)
        d["wxmid2"] = wx2.astype(NPBF16)
        for src, name in ((b_mid, "bmid"), (g_mid, "gmid"), (be_mid, "bemid")):
            d[name] = np.ascontiguousarray(
                src[MPC * c:MPC * (c + 1)].reshape(80, 1))
        d["wcroot"] = np.ascontiguousarray(
            W_root[GR + 80 * c:GR + 80 * (c + 1), :]).astype(NPBF16)
        d["wgroot"] = np.ascontiguousarray(
            W_root[16 * c:16 * (c + 1), :]).astype(NPBF16)
        d["xroot"] = np.ascontiguousarray(xrootT[16 * c:16 * (c + 1), :])
        for src, name in ((b_root, "broot"), (g_root, "groot"),
                          (be_root, "beroot")):
            d[name] = np.ascontiguousarray(src.reshape(OR, 1))
        in_maps.append(d)
    return in_maps


_NC_CACHE = {}


def _get_nc():
    if "nc" not in _NC_CACHE:
        _NC_CACHE["nc"] = _build_nc()
    return _NC_CACHE["nc"]


def kernel(**inputs) -> np.ndarray:
    nc = _get_nc()
    in_maps = _prep_in_maps(inputs)
    res = bass_utils.run_bass_kernel_spmd(
        nc, in_maps, core_ids=list(range(NCORES)))
    out = res.results[0]["y"]                                   # [38, 2048]
    return np.ascontiguousarray(out.T).astype(np.float32)       # [2048, 38]


# revision 21
# speedup vs baseline: 1.0208x; 1.0208x over previous
"""Trainium2 Bass kernel for nn_DCell (hierarchical DCell-style GNN).

Sharding: subsystem-parallel across 8 NeuronCores. Each core owns 64 of the
512 leaf subsystems (16 groups of 4 leaves, block-diagonal matmuls with
K=128) and the 4 mid subsystems fed by exactly those leaves. BatchNorm batch
stats (full batch B=2048) are therefore fully local to a core for the leaf
and mid layers; the BN affine is folded into the *next* layer's weights so
no full-size normalization pass over activations is ever needed. The root
layer is computed as per-core partial pre-activations (each core contributes
its 4 mids' features + a 16-row slice of the root gene input) that are
summed with a single AllReduce; every core then redundantly finishes the
root (tanh + full-batch BN) on the small [38, 2048] tensor.

Compute dtype is bf16 (inputs cast on host -> half the HBM traffic, full
TensorE rate); all normalization statistics and folds are fp32.

kernel(**inputs) takes full unsharded inputs, returns the full [2048, 38]
float32 output.
"""

import ml_dtypes
import numpy as np

import concourse.bass as bass
import concourse.mybir as mybir
import concourse.tile as tile
from concourse import bacc
from concourse import bass_utils

# Problem constants (hardcoded; kernel.py must be self-contained)
S, B, GL, OL = 512, 2048, 32, 20
M, C, GM, OM = 32, 16, 64, 20
GR, OR = 128, 38
EPS = 1e-5
NCORES = 8
LPC = S // NCORES      # 64 leaves per core
GPC = LPC // 4         # 16 leaf groups of 4 per core
MPC = M // NCORES      # 4 mids per core
BT = 512               # batch tile (free dim per matmul / psum bank)
NBT = B // BT          # 4

f32 = mybir.dt.float32
bf16 = mybir.dt.bfloat16
i32 = mybir.dt.int32
AF = mybir.ActivationFunctionType
ALU = mybir.AluOpType
NPBF16 = ml_dtypes.bfloat16

MAGIC = 0x5F3759DF  # fast inverse sqrt seed


def _emit_rsqrt(nc, sp, tag, out, a, magic_t, n):
    """out = 1/sqrt(a) elementwise on [P, n] fp32 tiles, DVE-only.

    Quake magic seed + 2 Newton iterations: rel err ~5e-6. a > 0.
    """
    P = a.shape[0]
    sh = sp.tile([P, n], i32, tag=f"{tag}sh", name=f"{tag}sh")
    nc.vector.tensor_scalar(sh, a.bitcast(i32), 1, None,
                            ALU.arith_shift_right)
    y0 = sp.tile([P, n], i32, tag=f"{tag}y0", name=f"{tag}y0")
    nc.vector.tensor_tensor(y0, magic_t[:P, 0:n], sh, ALU.subtract)
    y = y0.bitcast(f32)
    for it in range(2):
        # y <- y * (1.5 - 0.5*a*y*y)
        t1 = sp.tile([P, n], f32, tag=f"{tag}t1", name=f"{tag}t1_{it}")
        nc.vector.tensor_mul(t1, y, y)                    # y^2
        t2 = sp.tile([P, n], f32, tag=f"{tag}t2", name=f"{tag}t2_{it}")
        nc.vector.tensor_mul(t2, a, t1)                   # a*y^2
        t3 = sp.tile([P, n], f32, tag=f"{tag}t3", name=f"{tag}t3_{it}")
        nc.vector.tensor_scalar(t3, t2, -0.5, 1.5, ALU.mult, ALU.add)
        dst = out if it == 1 else sp.tile([P, n], f32, tag=f"{tag}y",
                                          name=f"{tag}y_{it}")
        nc.vector.tensor_mul(dst, y, t3)
        y = dst


def _build_nc():
    """Build (once) the SPMD Bass program run identically on all 8 cores."""
    nc = bacc.Bacc(
        "TRN2",
        target_bir_lowering=False,
        debug=False,
        enable_asserts=False,
        num_devices=NCORES,
    )

    # ---- per-core external I/O ----
    xleaf = nc.dram_tensor("xleaf", [GPC, 128, B], bf16, kind="ExternalInput").ap()
    wleaf = nc.dram_tensor("wleaf", [128, GPC * 80], bf16, kind="ExternalInput").ap()
    bleaf = nc.dram_tensor("bleaf", [80, GPC], f32, kind="ExternalInput").ap()
    gleaf = nc.dram_tensor("gleaf", [80, GPC], f32, kind="ExternalInput").ap()
    beleaf = nc.dram_tensor("beleaf", [80, GPC], f32, kind="ExternalInput").ap()
    xmid2 = nc.dram_tensor("xmid2", [2, 128, B], bf16, kind="ExternalInput").ap()
    wgmid = nc.dram_tensor("wgmid", [80, GPC * 80], bf16, kind="ExternalInput").ap()
    wxmid2 = nc.dram_tensor("wxmid2", [128, 2 * 80], bf16, kind="ExternalInput").ap()
    bmid = nc.dram_tensor("bmid", [80, 1], f32, kind="ExternalInput").ap()
    gmid = nc.dram_tensor("gmid", [80, 1], f32, kind="ExternalInput").ap()
    bemid = nc.dram_tensor("bemid", [80, 1], f32, kind="ExternalInput").ap()
    wcroot = nc.dram_tensor("wcroot", [80, OR], bf16, kind="ExternalInput").ap()
    wgroot = nc.dram_tensor("wgroot", [16, OR], bf16, kind="ExternalInput").ap()
    xroot = nc.dram_tensor("xroot", [16, B], bf16, kind="ExternalInput").ap()
    broot = nc.dram_tensor("broot", [OR, 1], f32, kind="ExternalInput").ap()
    groot = nc.dram_tensor("groot", [OR, 1], f32, kind="ExternalInput").ap()
    beroot = nc.dram_tensor("beroot", [OR, 1], f32, kind="ExternalInput").ap()
    y = nc.dram_tensor("y", [OR, B], f32, kind="ExternalOutput").ap()

    with tile.TileContext(nc) as tc:
        with (
            tc.tile_pool(name="const", bufs=1) as cp,
            tc.tile_pool(name="xp", bufs=8) as xp,
            tc.tile_pool(name="lt", bufs=16) as ltp,
            tc.tile_pool(name="small", bufs=2) as sp,
            tc.tile_pool(name="big", bufs=1) as bp,
            tc.tile_pool(name="psA", bufs=3, space="PSUM") as psA,
            tc.tile_pool(name="psM", bufs=4, space="PSUM") as psM,
            tc.tile_pool(name="psS", bufs=1, space="PSUM") as psS,
            tc.tile_pool(name="dram", bufs=1, space="DRAM") as dp,
        ):
            # ---- load constants/weights into SBUF ----
            wleaf_sb = cp.tile_from(wleaf)
            bleaf_sb = cp.tile_from(bleaf)
            gleaf_sb = cp.tile_from(gleaf)
            beleaf_sb = cp.tile_from(beleaf)
            wgmid_sb = cp.tile_from(wgmid)
            wxmid2_sb = cp.tile_from(wxmid2)
            xmid2a_sb = cp.tile_from(xmid2[0])
            xmid2b_sb = cp.tile_from(xmid2[1])
            xmid2_sb = [xmid2a_sb, xmid2b_sb]
            bmid_sb = cp.tile_from(bmid)
            gmid_sb = cp.tile_from(gmid)
            bemid_sb = cp.tile_from(bemid)
            wcroot_sb = cp.tile_from(wcroot)
            wgroot_sb = cp.tile_from(wgroot)
            xroot_sb = cp.tile_from(xroot)
            broot_sb = cp.tile_from(broot)
            groot_sb = cp.tile_from(groot)
            beroot_sb = cp.tile_from(beroot)

            magic_t = cp.tile([80, 4], i32, tag="magic", name="magict")
            nc.vector.memset(magic_t, MAGIC)
            zbias = cp.tile([80, 1], f32, tag="zbias", name="zbias")
            nc.vector.memset(zbias, 0.0)

            # persistent mid-accumulation psum banks (one per batch tile)
            mid_ps = [psM.tile([80, BT], f32, tag="mid", name=f"midps{b}")
                      for b in range(NBT)]
            # u = sum over all child features f of W_mid[f,:] * t_f (packed)
            u_ps = psS.tile([80, 1], f32, tag="tiny", name="ups")

            lt_tiles = []
            mv_cols = cp.tile([80, GPC, 2], f32, tag="mvall", name="mvall")
            stats_t = None
            for gi in range(GPC):
                mi, gj = gi // 4, gi % 4
                act_stats = (gi >= 8)       # later groups: stats via ACT accum
                ltile = ltp.tile([80, B], bf16, tag="lt", name=f"lt{gi}")
                lt_tiles.append(ltile)
                if gj == 0 and gi < 8:
                    stats_t = sp.tile([80, 4, NBT, 6], f32, tag="st",
                                      name=f"st{mi}")
                if act_stats:
                    sumx = sp.tile([80, NBT], f32, tag="sx", name=f"sx{gi}")
                    sumq = sp.tile([80, NBT], f32, tag="sq", name=f"sq{gi}")
                # ---- leaf matmul + tanh (+ Sum accum) per batch tile ----
                xt = xp.tile([128, B], bf16, tag="x", name=f"x{gi}")
                nc.sync.dma_start(out=xt, in_=xleaf[gi])
                for bt in range(NBT):
                    ps = psA.tile([80, BT], f32, tag="leaf",
                                  name=f"lfps{gi}_{bt}")
                    nc.tensor.matmul(
                        ps[:, :],
                        wleaf_sb[:, 80 * gi:80 * gi + 80],
                        xt[:, bt * BT:(bt + 1) * BT],
                        start=True, stop=True)
                    nc.scalar.activation(
                        ltile[:, bt * BT:(bt + 1) * BT], ps[:, :],
                        AF.Tanh, bias=bleaf_sb[:, gi:gi + 1], scale=1.0,
                        accum_out=sumx[:, bt:bt + 1] if act_stats else None)
                # ---- leaf BN stats for this group ----
                if act_stats:
                    sqs = sp.tile([80, BT], bf16, tag="sqs", name=f"sqs{gi}")
                    for bt in range(NBT):
                        nc.scalar.activation(
                            sqs[:, :], ltile[:, bt * BT:(bt + 1) * BT],
                            AF.Square, bias=zbias[:, 0:1],
                            accum_out=sumq[:, bt:bt + 1])
                    # mean = sum(sumx)/B ; var = sum(sumq)/B - mean^2
                    sxr = sp.tile([80, 1], f32, tag="sxr", name=f"sxr{gi}")
                    nc.vector.tensor_reduce(
                        out=sxr, in_=sumx[:, :],
                        op=ALU.add, axis=mybir.AxisListType.X)
                    nc.vector.tensor_scalar_mul(
                        mv_cols[:, gi, 0:1], sxr, 1.0 / B)
                    mq = sp.tile([80, 1], f32, tag="mq2", name=f"mq2{gi}")
                    nc.vector.tensor_mul(mq, mv_cols[:, gi, 0:1],
                                         mv_cols[:, gi, 0:1])
                    sqr = sp.tile([80, 1], f32, tag="sqr", name=f"sqr{gi}")
                    nc.vector.tensor_reduce(
                        out=sqr, in_=sumq[:, :],
                        op=ALU.add, axis=mybir.AxisListType.X)
                    nc.vector.tensor_scalar(
                        mv_cols[:, gi, 1:2], sqr, 1.0 / B, mq,
                        ALU.mult, ALU.subtract)
                else:
                    for bt in range(NBT):
                        nc.vector.bn_stats(
                            out=stats_t[:, gj, bt, :],
                            in_=ltile[:, bt * BT:(bt + 1) * BT])
                    nc.vector.bn_aggr(out=mv_cols[:, gi, :],
                                      in_=stats_t[:, gj, :, :])

                if gj != 3:
                    continue
                # ---- mid mi: fold BN into child weights, accumulate ----
                mean4 = mv_cols[:, 4 * mi:4 * mi + 4, 0]
                var4 = mv_cols[:, 4 * mi:4 * mi + 4, 1]
                a4 = sp.tile([80, 4], f32, tag="a4", name=f"a4{mi}")
                nc.vector.tensor_scalar_add(a4, var4, EPS)
                rs4 = sp.tile([80, 4], f32, tag="rs4", name=f"rs4{mi}")
                _emit_rsqrt(nc, sp, "lf", rs4, a4, magic_t, 4)
                s_t = sp.tile([80, 4], f32, tag="s", name=f"s{mi}")
                nc.vector.tensor_mul(s_t, gleaf_sb[:, 4 * mi:4 * mi + 4], rs4)
                ms = sp.tile([80, 4], f32, tag="ms", name=f"ms{mi}")
                nc.vector.tensor_mul(ms, mean4, s_t)
                t_t = sp.tile([80, 4], bf16, tag="t", name=f"t{mi}")
                nc.vector.tensor_sub(t_t, beleaf_sb[:, 4 * mi:4 * mi + 4], ms)

                wft = sp.tile([80, 4 * 80], bf16, tag="wf", name=f"wf{mi}")
                for g2 in range(4):
                    idx = 4 * mi + g2
                    nc.vector.tensor_scalar_mul(
                        wft[:, 80 * g2:80 * g2 + 80],
                        wgmid_sb[:, 80 * idx:80 * idx + 80],
                        s_t[:, g2:g2 + 1])
                    nc.tensor.matmul(
                        u_ps[:, :],
                        wgmid_sb[:, 80 * idx:80 * idx + 80],
                        t_t[:, g2:g2 + 1],
                        start=(mi == 0 and g2 == 0),
                        stop=(mi == 3 and g2 == 3))
                for bt in range(NBT):
                    for g2 in range(4):
                        idx = 4 * mi + g2
                        nc.tensor.matmul(
                            mid_ps[bt][:, :],
                            wft[:, 80 * g2:80 * g2 + 80],
                            lt_tiles[idx][:, bt * BT:(bt + 1) * BT],
                            start=(mi == 0 and g2 == 0), stop=False)
                    if mi % 2 == 1:  # gene blocks for mid pair (mi-1, mi)
                        pr = mi // 2
                        nc.tensor.matmul(
                            mid_ps[bt][:, :],
                            wxmid2_sb[:, 80 * pr:80 * pr + 80],
                            xmid2_sb[pr][:, bt * BT:(bt + 1) * BT],
                            start=False, stop=(mi == 3))

            # ---- mid finish: bias (b_mid + u), tanh, BN stats ----
            vmid = sp.tile([80, 1], f32, tag="vmid", name="vmid")
            nc.vector.tensor_add(vmid, u_ps[:, :], bmid_sb[:, :])
            tmt = bp.tile([80, B], bf16, tag="tm", name="tmt")
            mst = sp.tile([80, NBT, 6], f32, tag="mst", name="mst")
            for bt in range(NBT):
                nc.scalar.activation(
                    tmt[:, bt * BT:(bt + 1) * BT], mid_ps[bt][:, :],
                    AF.Tanh, bias=vmid[:, 0:1], scale=1.0)
                nc.vector.bn_stats(out=mst[:, bt, :],
                                   in_=tmt[:, bt * BT:(bt + 1) * BT])
            mmv = sp.tile([80, 2], f32, tag="mmv", name="mmv")
            nc.vector.bn_aggr(out=mmv[:, :], in_=mst[:, :, :])
            ma = sp.tile([80, 1], f32, tag="ma", name="ma")
            nc.vector.tensor_scalar_add(ma, mmv[:, 1:2], EPS)
            mrs = sp.tile([80, 1], f32, tag="mrs", name="mrs")
            _emit_rsqrt(nc, sp, "md", mrs, ma, magic_t, 1)
            msm = sp.tile([80, 1], f32, tag="msm", name="msm")
            nc.vector.tensor_mul(msm, gmid_sb[:, :], mrs)
            mms = sp.tile([80, 1], f32, tag="mms", name="mms")
            nc.vector.tensor_mul(mms, mmv[:, 0:1], msm)
            mtm = sp.tile([80, 1], bf16, tag="mtm", name="mtm")
            nc.vector.tensor_sub(mtm, bemid_sb[:, :], mms)

            # fold mid BN into root child-weight block; const vec v_c
            wcf = sp.tile([80, OR], bf16, tag="wcf", name="wcf")
            nc.vector.tensor_scalar_mul(wcf, wcroot_sb[:, :], msm[:, 0:1])
            v_ps = psS.tile([OR, 1], f32, tag="tiny", name="vps")
            nc.tensor.matmul(v_ps[:, :], wcroot_sb[:, :], mtm[:, 0:1],
                             start=True, stop=True)
            v_sb = sp.tile([OR, 1], f32, tag="vsb", name="vsb")
            nc.vector.tensor_copy(v_sb, v_ps[:, :])

            # ---- root partial pre-activation [38, B] ----
            partial = bp.tile([OR, B], bf16, tag="prt", name="partial")
            for bt in range(NBT):
                psr = psA.tile([OR, BT], f32, tag="leaf", name=f"rtps{bt}")
                nc.tensor.matmul(
                    psr[:, :], wcf[:, :],
                    tmt[:, bt * BT:(bt + 1) * BT],
                    start=True, stop=False)
                nc.tensor.matmul(
                    psr[:, :], wgroot_sb[:, :],
                    xroot_sb[:, bt * BT:(bt + 1) * BT],
                    start=False, stop=True)
                nc.vector.tensor_scalar_add(
                    partial[:, bt * BT:(bt + 1) * BT], psr[:, :],
                    v_sb[:, 0:1])

            # ---- AllReduce the partial root pre-activation (bf16) ----
            cc_in = dp.tile([OR, B], bf16, tag="ccin", name="ccin")
            cc_out = dp.tile([OR, B], bf16, tag="ccout", name="ccout")
            nc.sync.dma_start(out=cc_in[:, :], in_=partial[:, :])
            nc.gpsimd.collective_compute(
                "AllReduce",
                ALU.add,
                replica_groups=[list(range(NCORES))],
                ins=[cc_in.opt()],
                outs=[cc_out.opt()],
            )
            rsum = bp.tile([OR, B], bf16, tag="prt2", name="rsum")
            nc.sync.dma_start(out=rsum[:, :], in_=cc_out[:, :])

            # ---- root finish: tanh, full-batch BN, write out ----
            rt = bp.tile([OR, B], bf16, tag="rt", name="rt")
            nc.scalar.activation(rt[:, :], rsum[:, :], AF.Tanh,
                                 bias=broot_sb[:, 0:1], scale=1.0)
            rst = sp.tile([OR, NBT, 6], f32, tag="rst", name="rst")
            for bt in range(NBT):
                nc.vector.bn_stats(out=rst[:, bt, :],
                                   in_=rt[:, bt * BT:(bt + 1) * BT])
            rmv = sp.tile([OR, 2], f32, tag="rmv", name="rmv")
            nc.vector.bn_aggr(out=rmv[:, :], in_=rst[:, :, :])
            ra = sp.tile([OR, 1], f32, tag="ra", name="ra")
            nc.vector.tensor_scalar_add(ra, rmv[:, 1:2], EPS)
            rrs = sp.tile([OR, 1], f32, tag="rrs", name="rrs")
            _emit_rsqrt(nc, sp, "rt", rrs, ra, magic_t, 1)
            rsc = sp.tile([OR, 1], f32, tag="rsc", name="rsc")
            nc.vector.tensor_mul(rsc, groot_sb[:, :], rrs)
            rms = sp.tile([OR, 1], f32, tag="rms", name="rms")
            nc.vector.tensor_mul(rms, rmv[:, 0:1], rsc)
            rsh = sp.tile([OR, 1], f32, tag="rsh", name="rsh")
            nc.vector.tensor_sub(rsh, beroot_sb[:, :], rms)
            ysb = bp.tile([OR, B], f32, tag="ysb", name="ysb")
            nc.vector.tensor_scalar(ysb[:, :], rt[:, :], rsc[:, 0:1],
                                    rsh[:, 0:1], ALU.mult, ALU.add)
            nc.sync.dma_start(out=y, in_=ysb[:, :])

    nc.compile()
    return nc


def _prep_in_maps(inputs):
    """Host-side sharding + layout prep (incl. bf16 cast). 8 in_maps."""
    f = np.float32
    x_leaf = np.asarray(inputs["x_leaf"], dtype=f)
    x_mid = np.asarray(inputs["x_mid"], dtype=f)
    x_root = np.asarray(inputs["x_root"], dtype=f)
    W_leaf = np.asarray(inputs["W_leaf"], dtype=f)
    b_leaf = np.asarray(inputs["b_leaf"], dtype=f)
    g_leaf = np.asarray(inputs["g_leaf"], dtype=f)
    be_leaf = np.asarray(inputs["be_leaf"], dtype=f)
    W_mid = np.asarray(inputs["W_mid"], dtype=f)
    b_mid = np.asarray(inputs["b_mid"], dtype=f)
    g_mid = np.asarray(inputs["g_mid"], dtype=f)
    be_mid = np.asarray(inputs["be_mid"], dtype=f)
    W_root = np.asarray(inputs["W_root"], dtype=f)
    b_root = np.asarray(inputs["b_root"], dtype=f)
    g_root = np.asarray(inputs["g_root"], dtype=f)
    be_root = np.asarray(inputs["be_root"], dtype=f)

    # gene-major leaf inputs, 4 leaves stacked per 128-partition group
    xleafT = np.ascontiguousarray(
        x_leaf.reshape(NCORES, GPC, 4, B, GL).transpose(0, 1, 2, 4, 3)
        .reshape(NCORES, GPC, 128, B)).astype(NPBF16)
    # mid gene inputs: per core, mid pairs (0,1) and (2,3) stacked to 128
    xmidT = (x_mid.reshape(NCORES, 2, 2, B, GM).transpose(0, 1, 2, 4, 3)
             .reshape(NCORES, 2, 128, B)).astype(NPBF16)
    xrootT = np.ascontiguousarray(x_root.T).astype(NPBF16)     # [128, B]

    in_maps = []
    for c in range(NCORES):
        d = {}
        d["xleaf"] = np.ascontiguousarray(xleafT[c])
        # block-diagonal leaf weights [128, 16*80]
        wl = np.zeros((128, GPC * 80), f)
        for gi in range(GPC):
            for j in range(4):
                s = LPC * c + 4 * gi + j
                wl[32 * j:32 * j + 32,
                   80 * gi + 20 * j:80 * gi + 20 * j + 20] = W_leaf[s]
        d["wleaf"] = wl.astype(NPBF16)
        for src, name in ((b_leaf, "bleaf"), (g_leaf, "gleaf"),
                          (be_leaf, "beleaf")):
            d[name] = np.ascontiguousarray(
                src[LPC * c:LPC * (c + 1)].reshape(GPC, 80).T)
        d["xmid2"] = np.ascontiguousarray(xmidT[c])
        wg = np.zeros((80, GPC * 80), f)
        # gene blocks for mid pairs: [128, 2*80]
        wx2 = np.zeros((128, 2 * 80), f)
        for mi in range(MPC):
            m = MPC * c + mi
            for gj in range(4):
                idx = 4 * mi + gj
                wg[:, 80 * idx + 20 * mi:80 * idx + 20 * mi + 20] = \
                    W_mid[m, GM + 80 * gj:GM + 80 * gj + 80, :]
            pr, sub = mi // 2, mi % 2
            wx2[64 * sub:64 * sub + 64,
                80 * pr + 20 * mi:80 * pr + 20 * mi + 20] = W_mid[m, :GM, :]
        d["wgmid"] = wg.astype(NPBF16)
        d["wxmid2"] = wx2.astype(NPBF16)
        for src, name in ((b_mid, "bmid"), (g_mid, "gmid"), (be_mid, "bemid")):
            d[name] = np.ascontiguousarray(
                src[MPC * c:MPC * (c + 1)].reshape(80, 1))
        d["wcroot"] = np.ascontiguousarray(
            W_root[GR + 80 * c:GR + 80 * (c + 1), :]).astype(NPBF16)
        d["wgroot"] = np.ascontiguousarray(
            W_root[16 * c:16 * (c + 1), :]).astype(NPBF16)
        d["xroot"] = np.ascontiguousarray(xrootT[16 * c:16 * (c + 1), :])
        for src, name in ((b_root, "broot"), (g_root, "groot"),
                          (be_root, "beroot")):
            d[name] = np.ascontiguousarray(src.reshape(OR, 1))
        in_maps.append(d)
    return in_maps


_NC_CACHE = {}


def _get_nc():
    if "nc" not in _NC_CACHE:
        _NC_CACHE["nc"] = _build_nc()
    return _NC_CACHE["nc"]


def kernel(**inputs) -> np.ndarray:
    nc = _get_nc()
    in_maps = _prep_in_maps(inputs)
    res = bass_utils.run_bass_kernel_spmd(
        nc, in_maps, core_ids=list(range(NCORES)))
    out = res.results[0]["y"]                                   # [38, 2048]
    return np.ascontiguousarray(out.T).astype(np.float32)       # [2048, 38]


# revision 22
# speedup vs baseline: 1.1620x; 1.1383x over previous
"""Trainium2 Bass kernel for nn_DCell (hierarchical DCell-style GNN).

Sharding: subsystem-parallel across 8 NeuronCores. Each core owns 64 of the
512 leaf subsystems (16 groups of 4 leaves, block-diagonal matmuls with
K=128) and the 4 mid subsystems fed by exactly those leaves. BatchNorm batch
stats (full batch B=2048) are therefore fully local to a core for the leaf
and mid layers; the BN affine is folded into the *next* layer's weights so
no full-size normalization pass over activations is ever needed. The root
layer is computed as per-core partial pre-activations (each core contributes
its 4 mids' features + a 16-row slice of the root gene input) that are
summed with a single AllReduce; every core then redundantly finishes the
root (tanh + full-batch BN) on the small [38, 2048] tensor.

Compute dtype is bf16 (inputs cast on host -> half the HBM traffic, full
TensorE rate); all normalization statistics and folds are fp32.

kernel(**inputs) takes full unsharded inputs, returns the full [2048, 38]
float32 output.
"""

import ml_dtypes
import numpy as np

import concourse.bass as bass
import concourse.mybir as mybir
import concourse.tile as tile
from concourse import bacc
from concourse import bass_utils

# Problem constants (hardcoded; kernel.py must be self-contained)
S, B, GL, OL = 512, 2048, 32, 20
M, C, GM, OM = 32, 16, 64, 20
GR, OR = 128, 38
EPS = 1e-5
NCORES = 8
LPC = S // NCORES      # 64 leaves per core
GPC = LPC // 4         # 16 leaf groups of 4 per core
MPC = M // NCORES      # 4 mids per core
BT = 512               # batch tile (free dim per matmul / psum bank)
NBT = B // BT          # 4

f32 = mybir.dt.float32
bf16 = mybir.dt.bfloat16
i32 = mybir.dt.int32
AF = mybir.ActivationFunctionType
ALU = mybir.AluOpType
NPBF16 = ml_dtypes.bfloat16

MAGIC = 0x5F3759DF  # fast inverse sqrt seed


def _emit_rsqrt(nc, sp, tag, out, a, magic_t, n):
    """out = 1/sqrt(a) elementwise on [P, n] fp32 tiles, DVE-only.

    Quake magic seed + 2 Newton iterations: rel err ~5e-6. a > 0.
    """
    P = a.shape[0]
    sh = sp.tile([P, n], i32, tag=f"{tag}sh", name=f"{tag}sh")
    nc.vector.tensor_scalar(sh, a.bitcast(i32), 1, None,
                            ALU.arith_shift_right)
    y0 = sp.tile([P, n], i32, tag=f"{tag}y0", name=f"{tag}y0")
    nc.vector.tensor_tensor(y0, magic_t[:P, 0:n], sh, ALU.subtract)
    y = y0.bitcast(f32)
    for it in range(2):
        # y <- y * (1.5 - 0.5*a*y*y)
        t1 = sp.tile([P, n], f32, tag=f"{tag}t1", name=f"{tag}t1_{it}")
        nc.vector.tensor_mul(t1, y, y)                    # y^2
        t2 = sp.tile([P, n], f32, tag=f"{tag}t2", name=f"{tag}t2_{it}")
        nc.vector.tensor_mul(t2, a, t1)                   # a*y^2
        t3 = sp.tile([P, n], f32, tag=f"{tag}t3", name=f"{tag}t3_{it}")
        nc.vector.tensor_scalar(t3, t2, -0.5, 1.5, ALU.mult, ALU.add)
        dst = out if it == 1 else sp.tile([P, n], f32, tag=f"{tag}y",
                                          name=f"{tag}y_{it}")
        nc.vector.tensor_mul(dst, y, t3)
        y = dst


def _build_nc():
    """Build (once) the SPMD Bass program run identically on all 8 cores."""
    nc = bacc.Bacc(
        "TRN2",
        target_bir_lowering=False,
        debug=False,
        enable_asserts=False,
        num_devices=NCORES,
    )

    # ---- per-core external I/O ----
    xleaf = nc.dram_tensor("xleaf", [GPC, 128, B], bf16, kind="ExternalInput").ap()
    wleaf = nc.dram_tensor("wleaf", [128, GPC * 80], bf16, kind="ExternalInput").ap()
    bleaf = nc.dram_tensor("bleaf", [80, GPC], f32, kind="ExternalInput").ap()
    gleaf = nc.dram_tensor("gleaf", [80, GPC], f32, kind="ExternalInput").ap()
    beleaf = nc.dram_tensor("beleaf", [80, GPC], f32, kind="ExternalInput").ap()
    xmid2 = nc.dram_tensor("xmid2", [2, 128, B], bf16, kind="ExternalInput").ap()
    wgmid = nc.dram_tensor("wgmid", [80, GPC * 80], bf16, kind="ExternalInput").ap()
    wxmid2 = nc.dram_tensor("wxmid2", [128, 2 * 80], bf16, kind="ExternalInput").ap()
    bmid = nc.dram_tensor("bmid", [80, 1], f32, kind="ExternalInput").ap()
    gmid = nc.dram_tensor("gmid", [80, 1], f32, kind="ExternalInput").ap()
    bemid = nc.dram_tensor("bemid", [80, 1], f32, kind="ExternalInput").ap()
    wcroot = nc.dram_tensor("wcroot", [80, OR], bf16, kind="ExternalInput").ap()
    wgroot = nc.dram_tensor("wgroot", [16, OR], bf16, kind="ExternalInput").ap()
    xroot = nc.dram_tensor("xroot", [16, B], bf16, kind="ExternalInput").ap()
    broot = nc.dram_tensor("broot", [OR, 1], f32, kind="ExternalInput").ap()
    groot = nc.dram_tensor("groot", [OR, 1], f32, kind="ExternalInput").ap()
    beroot = nc.dram_tensor("beroot", [OR, 1], f32, kind="ExternalInput").ap()
    y = nc.dram_tensor("y", [OR, B], f32, kind="ExternalOutput").ap()

    with tile.TileContext(nc) as tc:
        with (
            tc.tile_pool(name="const", bufs=1) as cp,
            tc.tile_pool(name="xp", bufs=8) as xp,
            tc.tile_pool(name="lt", bufs=16) as ltp,
            tc.tile_pool(name="small", bufs=2) as sp,
            tc.tile_pool(name="big", bufs=1) as bp,
            tc.tile_pool(name="psA", bufs=2, space="PSUM") as psA,
            tc.tile_pool(name="psM", bufs=4, space="PSUM") as psM,
            tc.tile_pool(name="dram", bufs=1, space="DRAM") as dp,
        ):
            # ---- load constants/weights into SBUF ----
            wleaf_sb = cp.tile_from(wleaf)
            bleaf_sb = cp.tile_from(bleaf)
            gleaf_sb = cp.tile_from(gleaf)
            beleaf_sb = cp.tile_from(beleaf)
            wgmid_sb = cp.tile_from(wgmid)
            wxmid2_sb = cp.tile_from(wxmid2)
            xmid2a_sb = cp.tile_from(xmid2[0])
            xmid2b_sb = cp.tile_from(xmid2[1])
            xmid2_sb = [xmid2a_sb, xmid2b_sb]
            bmid_sb = cp.tile_from(bmid)
            gmid_sb = cp.tile_from(gmid)
            bemid_sb = cp.tile_from(bemid)
            wcroot_sb = cp.tile_from(wcroot)
            wgroot_sb = cp.tile_from(wgroot)
            xroot_sb = cp.tile_from(xroot)
            broot_sb = cp.tile_from(broot)
            groot_sb = cp.tile_from(groot)
            beroot_sb = cp.tile_from(beroot)

            magic_t = cp.tile([80, 4], i32, tag="magic", name="magict")
            nc.vector.memset(magic_t, MAGIC)
            zbias = cp.tile([80, 1], f32, tag="zbias", name="zbias")
            nc.vector.memset(zbias, 0.0)

            # persistent mid-accumulation psum banks (one per batch tile)
            mid_ps = [psM.tile([80, BT], f32, tag="mid", name=f"midps{b}")
                      for b in range(NBT)]

            lt_tiles = []
            mv_cols = cp.tile([80, GPC, 2], f32, tag="mvall", name="mvall")
            stats_t = None
            for gi in range(GPC):
                mi, gj = gi // 4, gi % 4
                act_stats = (gi >= 8)       # later groups: stats via ACT accum
                ltile = ltp.tile([80, B], bf16, tag="lt", name=f"lt{gi}")
                lt_tiles.append(ltile)
                if gj == 0 and gi < 8:
                    stats_t = sp.tile([80, 4, NBT, 6], f32, tag="st",
                                      name=f"st{mi}")
                if act_stats:
                    sumx = sp.tile([80, 2], f32, tag="sx", name=f"sx{gi}")
                    sumq = sp.tile([80, 2], f32, tag="sq", name=f"sq{gi}")
                # ---- leaf matmul + tanh (+ Sum accum) per half batch ----
                xt = xp.tile([128, B], bf16, tag="x", name=f"x{gi}")
                nc.sync.dma_start(out=xt, in_=xleaf[gi])
                for h in range(2):
                    ps = psA.tile([80, 2 * BT], f32, tag="leaf",
                                  name=f"lfps{gi}_{h}")
                    for s2 in range(2):
                        nc.tensor.matmul(
                            ps[:, s2 * BT:(s2 + 1) * BT],
                            wleaf_sb[:, 80 * gi:80 * gi + 80],
                            xt[:, (2 * h + s2) * BT:(2 * h + s2 + 1) * BT],
                            start=True, stop=True)
                    nc.scalar.activation(
                        ltile[:, 2 * h * BT:2 * (h + 1) * BT], ps[:, :],
                        AF.Tanh, bias=bleaf_sb[:, gi:gi + 1], scale=1.0,
                        accum_out=sumx[:, h:h + 1] if act_stats else None)
                # ---- leaf BN stats for this group ----
                if act_stats:
                    sqs = sp.tile([80, 2 * BT], bf16, tag="sqs",
                                  name=f"sqs{gi}")
                    for h in range(2):
                        nc.scalar.activation(
                            sqs[:, :], ltile[:, 2 * h * BT:2 * (h + 1) * BT],
                            AF.Square, bias=zbias[:, 0:1],
                            accum_out=sumq[:, h:h + 1])
                    # mean = sum(sumx)/B ; var = sum(sumq)/B - mean^2
                    sxr = sp.tile([80, 1], f32, tag="sxr", name=f"sxr{gi}")
                    nc.vector.tensor_reduce(
                        out=sxr, in_=sumx[:, :],
                        op=ALU.add, axis=mybir.AxisListType.X)
                    nc.vector.tensor_scalar_mul(
                        mv_cols[:, gi, 0:1], sxr, 1.0 / B)
                    mq = sp.tile([80, 1], f32, tag="mq2", name=f"mq2{gi}")
                    nc.vector.tensor_mul(mq, mv_cols[:, gi, 0:1],
                                         mv_cols[:, gi, 0:1])
                    sqr = sp.tile([80, 1], f32, tag="sqr", name=f"sqr{gi}")
                    nc.vector.tensor_reduce(
                        out=sqr, in_=sumq[:, :],
                        op=ALU.add, axis=mybir.AxisListType.X)
                    nc.vector.tensor_scalar(
                        mv_cols[:, gi, 1:2], sqr, 1.0 / B, mq,
                        ALU.mult, ALU.subtract)
                else:
                    for bt in range(NBT):
                        nc.vector.bn_stats(
                            out=stats_t[:, gj, bt, :],
                            in_=ltile[:, bt * BT:(bt + 1) * BT])
                    nc.vector.bn_aggr(out=mv_cols[:, gi, :],
                                      in_=stats_t[:, gj, :, :])

                if gj != 3:
                    continue
                # ---- mid mi: fold BN into child weights, accumulate ----
                mean4 = mv_cols[:, 4 * mi:4 * mi + 4, 0]
                var4 = mv_cols[:, 4 * mi:4 * mi + 4, 1]
                a4 = sp.tile([80, 4], f32, tag="a4", name=f"a4{mi}")
                nc.vector.tensor_scalar_add(a4, var4, EPS)
                rs4 = sp.tile([80, 4], f32, tag="rs4", name=f"rs4{mi}")
                _emit_rsqrt(nc, sp, "lf", rs4, a4, magic_t, 4)
                s_t = sp.tile([80, 4], f32, tag="s", name=f"s{mi}")
                nc.vector.tensor_mul(s_t, gleaf_sb[:, 4 * mi:4 * mi + 4], rs4)
                ms = sp.tile([80, 4], f32, tag="ms", name=f"ms{mi}")
                nc.vector.tensor_mul(ms, mean4, s_t)
                t_t = sp.tile([80, 4], f32, tag="t", name=f"t{mi}")
                nc.vector.tensor_sub(t_t, beleaf_sb[:, 4 * mi:4 * mi + 4], ms)

                # apply BN in place on the 4 bf16 leaf tiles (4x DVE mode)
                for g2 in range(4):
                    idx = 4 * mi + g2
                    nc.vector.tensor_scalar(
                        lt_tiles[idx][:, :], lt_tiles[idx][:, :],
                        s_t[:, g2:g2 + 1], t_t[:, g2:g2 + 1],
                        ALU.mult, ALU.add)
                for bt in range(NBT):
                    for g2 in range(4):
                        idx = 4 * mi + g2
                        nc.tensor.matmul(
                            mid_ps[bt][:, :],
                            wgmid_sb[:, 80 * idx:80 * idx + 80],
                            lt_tiles[idx][:, bt * BT:(bt + 1) * BT],
                            start=(mi == 0 and g2 == 0), stop=False)
                    if mi % 2 == 1:  # gene blocks for mid pair (mi-1, mi)
                        pr = mi // 2
                        nc.tensor.matmul(
                            mid_ps[bt][:, :],
                            wxmid2_sb[:, 80 * pr:80 * pr + 80],
                            xmid2_sb[pr][:, bt * BT:(bt + 1) * BT],
                            start=False, stop=(mi == 3))

            # ---- mid finish: tanh, BN stats ----
            tmt = bp.tile([80, B], bf16, tag="tm", name="tmt")
            mst = sp.tile([80, NBT, 6], f32, tag="mst", name="mst")
            for bt in range(NBT):
                nc.scalar.activation(
                    tmt[:, bt * BT:(bt + 1) * BT], mid_ps[bt][:, :],
                    AF.Tanh, bias=bmid_sb[:, 0:1], scale=1.0)
                nc.vector.bn_stats(out=mst[:, bt, :],
                                   in_=tmt[:, bt * BT:(bt + 1) * BT])
            mmv = sp.tile([80, 2], f32, tag="mmv", name="mmv")
            nc.vector.bn_aggr(out=mmv[:, :], in_=mst[:, :, :])
            ma = sp.tile([80, 1], f32, tag="ma", name="ma")
            nc.vector.tensor_scalar_add(ma, mmv[:, 1:2], EPS)
            mrs = sp.tile([80, 1], f32, tag="mrs", name="mrs")
            _emit_rsqrt(nc, sp, "md", mrs, ma, magic_t, 1)
            msm = sp.tile([80, 1], f32, tag="msm", name="msm")
            nc.vector.tensor_mul(msm, gmid_sb[:, :], mrs)
            mms = sp.tile([80, 1], f32, tag="mms", name="mms")
            nc.vector.tensor_mul(mms, mmv[:, 0:1], msm)
            mtm = sp.tile([80, 1], f32, tag="mtm", name="mtm")
            nc.vector.tensor_sub(mtm, bemid_sb[:, :], mms)
            # apply mid BN in place on tmt
            nc.vector.tensor_scalar(tmt[:, :], tmt[:, :], msm[:, 0:1],
                                    mtm[:, 0:1], ALU.mult, ALU.add)

            # ---- root partial pre-activation [38, B] ----
            partial = bp.tile([OR, B], bf16, tag="prt", name="partial")
            for bt in range(NBT):
                psr = psA.tile([OR, BT], f32, tag="leaf", name=f"rtps{bt}")
                nc.tensor.matmul(
                    psr[:, :], wcroot_sb[:, :],
                    tmt[:, bt * BT:(bt + 1) * BT],
                    start=True, stop=False)
                nc.tensor.matmul(
                    psr[:, :], wgroot_sb[:, :],
                    xroot_sb[:, bt * BT:(bt + 1) * BT],
                    start=False, stop=True)
                nc.vector.tensor_copy(
                    partial[:, bt * BT:(bt + 1) * BT], psr[:, :])

            # ---- AllReduce the partial root pre-activation (bf16) ----
            cc_in = dp.tile([OR, B], bf16, tag="ccin", name="ccin")
            cc_out = dp.tile([OR, B], bf16, tag="ccout", name="ccout")
            nc.sync.dma_start(out=cc_in[:, :], in_=partial[:, :])
            nc.gpsimd.collective_compute(
                "AllReduce",
                ALU.add,
                replica_groups=[list(range(NCORES))],
                ins=[cc_in.opt()],
                outs=[cc_out.opt()],
            )
            rsum = bp.tile([OR, B], bf16, tag="prt2", name="rsum")
            nc.sync.dma_start(out=rsum[:, :], in_=cc_out[:, :])

            # ---- root finish: tanh, full-batch BN, write out ----
            rt = bp.tile([OR, B], bf16, tag="rt", name="rt")
            nc.scalar.activation(rt[:, :], rsum[:, :], AF.Tanh,
                                 bias=broot_sb[:, 0:1], scale=1.0)
            rst = sp.tile([OR, NBT, 6], f32, tag="rst", name="rst")
            for bt in range(NBT):
                nc.vector.bn_stats(out=rst[:, bt, :],
                                   in_=rt[:, bt * BT:(bt + 1) * BT])
            rmv = sp.tile([OR, 2], f32, tag="rmv", name="rmv")
            nc.vector.bn_aggr(out=rmv[:, :], in_=rst[:, :, :])
            ra = sp.tile([OR, 1], f32, tag="ra", name="ra")
            nc.vector.tensor_scalar_add(ra, rmv[:, 1:2], EPS)
            rrs = sp.tile([OR, 1], f32, tag="rrs", name="rrs")
            _emit_rsqrt(nc, sp, "rt", rrs, ra, magic_t, 1)
            rsc = sp.tile([OR, 1], f32, tag="rsc", name="rsc")
            nc.vector.tensor_mul(rsc, groot_sb[:, :], rrs)
            rms = sp.tile([OR, 1], f32, tag="rms", name="rms")
            nc.vector.tensor_mul(rms, rmv[:, 0:1], rsc)
            rsh = sp.tile([OR, 1], f32, tag="rsh", name="rsh")
            nc.vector.tensor_sub(rsh, beroot_sb[:, :], rms)
            ysb = bp.tile([OR, B], f32, tag="ysb", name="ysb")
            nc.vector.tensor_scalar(ysb[:, :], rt[:, :], rsc[:, 0:1],
                                    rsh[:, 0:1], ALU.mult, ALU.add)
            nc.sync.dma_start(out=y, in_=ysb[:, :])

    nc.compile()
    return nc


def _prep_in_maps(inputs):
    """Host-side sharding + layout prep (incl. bf16 cast). 8 in_maps."""
    f = np.float32
    x_leaf = np.asarray(inputs["x_leaf"], dtype=f)
    x_mid = np.asarray(inputs["x_mid"], dtype=f)
    x_root = np.asarray(inputs["x_root"], dtype=f)
    W_leaf = np.asarray(inputs["W_leaf"], dtype=f)
    b_leaf = np.asarray(inputs["b_leaf"], dtype=f)
    g_leaf = np.asarray(inputs["g_leaf"], dtype=f)
    be_leaf = np.asarray(inputs["be_leaf"], dtype=f)
    W_mid = np.asarray(inputs["W_mid"], dtype=f)
    b_mid = np.asarray(inputs["b_mid"], dtype=f)
    g_mid = np.asarray(inputs["g_mid"], dtype=f)
    be_mid = np.asarray(inputs["be_mid"], dtype=f)
    W_root = np.asarray(inputs["W_root"], dtype=f)
    b_root = np.asarray(inputs["b_root"], dtype=f)
    g_root = np.asarray(inputs["g_root"], dtype=f)
    be_root = np.asarray(inputs["be_root"], dtype=f)

    # gene-major leaf inputs, 4 leaves stacked per 128-partition group
    xleafT = np.ascontiguousarray(
        x_leaf.reshape(NCORES, GPC, 4, B, GL).transpose(0, 1, 2, 4, 3)
        .reshape(NCORES, GPC, 128, B)).astype(NPBF16)
    # mid gene inputs: per core, mid pairs (0,1) and (2,3) stacked to 128
    xmidT = (x_mid.reshape(NCORES, 2, 2, B, GM).transpose(0, 1, 2, 4, 3)
             .reshape(NCORES, 2, 128, B)).astype(NPBF16)
    xrootT = np.ascontiguousarray(x_root.T).astype(NPBF16)     # [128, B]

    in_maps = []
    for c in range(NCORES):
        d = {}
        d["xleaf"] = np.ascontiguousarray(xleafT[c])
        # block-diagonal leaf weights [128, 16*80]
        wl = np.zeros((128, GPC * 80), f)
        for gi in range(GPC):
            for j in range(4):
                s = LPC * c + 4 * gi + j
                wl[32 * j:32 * j + 32,
                   80 * gi + 20 * j:80 * gi + 20 * j + 20] = W_leaf[s]
        d["wleaf"] = wl.astype(NPBF16)
        for src, name in ((b_leaf, "bleaf"), (g_leaf, "gleaf"),
                          (be_leaf, "beleaf")):
            d[name] = np.ascontiguousarray(
                src[LPC * c:LPC * (c + 1)].reshape(GPC, 80).T)
        d["xmid2"] = np.ascontiguousarray(xmidT[c])
        wg = np.zeros((80, GPC * 80), f)
        # gene blocks for mid pairs: [128, 2*80]
        wx2 = np.zeros((128, 2 * 80), f)
        for mi in range(MPC):
            m = MPC * c + mi
            for gj in range(4):
                idx = 4 * mi + gj
                wg[:, 80 * idx + 20 * mi:80 * idx + 20 * mi + 20] = \
                    W_mid[m, GM + 80 * gj:GM + 80 * gj + 80, :]
            pr, sub = mi // 2, mi % 2
            wx2[64 * sub:64 * sub + 64,
                80 * pr + 20 * mi:80 * pr + 20 * mi + 20] = W_mid[m, :GM, :]
        d["wgmid"] = wg.astype(NPBF16)
        d["wxmid2"] = wx2.astype(NPBF16)
        for src, name in ((b_mid, "bmid"), (g_mid, "gmid"), (be_mid, "bemid")):
            d[name] = np.ascontiguousarray(
                src[MPC * c:MPC * (c + 1)].reshape(80, 1))
        d["wcroot"] = np.ascontiguousarray(
            W_root[GR + 80 * c:GR + 80 * (c + 1), :]).astype(NPBF16)
        d["wgroot"] = np.ascontiguousarray(
            W_root[16 * c:16 * (c + 1), :]).astype(NPBF16)
        d["xroot"] = np.ascontiguousarray(xrootT[16 * c:16 * (c + 1), :])
        for src, name in ((b_root, "broot"), (g_root, "groot"),
                          (be_root, "beroot")):
            d[name] = np.ascontiguousarray(src.reshape(OR, 1))
        in_maps.append(d)
    return in_maps


_NC_CACHE = {}


def _get_nc():
    if "nc" not in _NC_CACHE:
        _NC_CACHE["nc"] = _build_nc()
    return _NC_CACHE["nc"]


def kernel(**inputs) -> np.ndarray:
    nc = _get_nc()
    in_maps = _prep_in_maps(inputs)
    res = bass_utils.run_bass_kernel_spmd(
        nc, in_maps, core_ids=list(range(NCORES)))
    out = res.results[0]["y"]                                   # [38, 2048]
    return np.ascontiguousarray(out.T).astype(np.float32)       # [2048, 38]
